# revision 37
# baseline (speedup 1.0000x reference)
"""GroupWiseLinear Trainium2 kernel.

out[b, c] = dot(W[0, c, :], x[b, group_of[c], :]) + bias[0, c], then a final
class-permutation gather, for two independent branches (co / cl).

Sharding: 8 cores = 2 branches x 4 class-ranges.  The ranges are chosen per
branch to BALANCE the per-core slot count (ragged group segments are split
into <=64-column slots; fixed quarters give 23..30 slots, balanced ranges
give <=26), which directly cuts the dominant DMA traffic.  Every core runs
the SAME instruction stream (SPMD) on different data:

  - xt:  [128, S*4*64]   per-slot x^T (H-major), replicated per slot
  - wt:  [128, 4*S*64]   W^T, zero-padded to slot layout, chunk-major so
                         every load chunk is one contiguous line/partition
  - bz:  [1, S*64]       bias, zero-padded to slot layout
  - o:   [64, S*64]      bf16 per-core output (batch-major)

Device work per slot: 4 K-chunk matmuls (x stationary [128,64], W moving)
accumulating into PSUM, closed by a per-psum-tile rank-1 bias matmul
(ones x bias row, so bias data is never on the early critical path).  PSUM
drains to SBUF as bf16: big tiles on the vector engine, the two small tail
tiles on the activation engine so they overlap.  Loads stream on one engine
in strict order as paired (wt, xt) chunks sized [4, 8, .., 4, 1, 1]; psum
tiles are [8, .., 8, 1, 1] and stores are {t0,t1} / {t2,t3} / {t4}, so the
chain after the last loaded byte (sem prop -> 5 matmuls -> small copy ->
tiny store -> sem prop -> exit barrier) is as short as the cost model
allows.  bz rides the Pool engine's SWDGE path to keep the shared HWDGE
descriptor generator (625-650 ns per DMA, serialized) feeding the loads.
"""

import ml_dtypes
import numpy as np

import concourse.bacc as bacc
import concourse.tile as tile
from concourse import mybir
from concourse.bass_utils import run_bass_kernel_spmd

B = 64          # batch
H = 512         # hidden
NC_CLS = 4096   # classes per branch
NQ = 4          # class-ranges per branch
KC = H // 128   # contraction chunks

_cache = {}


def _segments(go, lo, hi):
    """Maximal same-group runs of go within [lo, hi)."""
    segs = []
    i = lo
    while i < hi:
        g = go[i]
        j = i
        while j < hi and go[j] == g:
            j += 1
        segs.append((i, j, int(g)))
        i = j
    return segs


def _balanced_split(go, nparts=NQ):
    """Split [0, len(go)) into nparts ranges with near-equal slot counts."""
    segs = _segments(go, 0, len(go))
    total = sum((e - s + 63) // 64 for s, e, _ in segs)
    target = -(-total // nparts)
    cuts = [0]
    acc = 0
    for s, e, _ in segs:
        n = (e - s + 63) // 64
        while acc + n >= target and len(cuts) < nparts:
            take = target - acc
            cutpos = min(s + take * 64, e)
            cuts.append(cutpos)
            s = cutpos
            n = (e - s + 63) // 64
            acc = 0
        acc += n
    cuts.append(len(go))
    return list(zip(cuts[:-1], cuts[1:]))


def _build_shards(co_group_of, cl_group_of):
    """Per (branch, range): (r0, r1, slots) with slots (group, cls_start, w)."""
    shards = []
    for go in (co_group_of, cl_group_of):
        go = np.asarray(go).astype(np.int64)
        for r0, r1 in _balanced_split(go):
            slots = []
            for s0, s1, g in _segments(go, r0, r1):
                for s in range(s0, s1, 64):
                    slots.append((g, s, min(64, s1 - s)))
            shards.append((r0, r1, slots))
    return shards


def _chunk_bounds(S):
    """Slot-index bounds for paired (wt, xt) load chunks: a small first chunk
    so compute starts early, 8-slot middle chunks, and two single-slot final
    chunks so the tail of the pipeline is short."""
    bounds = [0, 4]
    while bounds[-1] < S - 2:
        bounds.append(min(bounds[-1] + 8, S - 2))
    bounds.append(S - 1)
    bounds.append(S)
    return bounds


def _ptile_bounds(S):
    """PSUM-tile bounds: 8 slots (one bank) each, single-slot final tile."""
    bounds = list(range(0, S - 1, 8))
    if bounds[-1] != S - 1:
        bounds.append(S - 1)
    bounds.append(S)
    return bounds


def _program(S, dt=mybir.dt.bfloat16):
    """Build the uniform SPMD Bass program for S slots per core."""
    nc = bacc.Bacc("TRN2", target_bir_lowering=False, debug=False, num_devices=8)
    xt_d = nc.dram_tensor("xt", [128, S * KC * 64], dt, kind="ExternalInput")
    wt_d = nc.dram_tensor("wt", [128, KC * S * 64], dt, kind="ExternalInput")
    bz_d = nc.dram_tensor("bz", [1, S * 64], dt, kind="ExternalInput")
    o_d = nc.dram_tensor("o", [64, S * 64], dt, kind="ExternalOutput")

    cb = _chunk_bounds(S)
    nch = len(cb) - 1
    pb = _ptile_bounds(S)
    npt = len(pb) - 1
    # store groups of psum tiles: pairs, with the small last tile alone so
    # the final store (and its whole chain) is tiny
    sgroups = []
    i = 0
    while i < npt - 1:
        j = min(i + 2, npt - 1)
        sgroups.append((i, j))
        i = j
    sgroups.append((npt - 1, npt))

    with tile.TileContext(nc) as tc:
        with (
            tc.tile_pool(name="bp", bufs=2) as bp,
            tc.tile_pool(name="wp", bufs=nch) as wp,
            tc.tile_pool(name="xp", bufs=nch) as xp,
            tc.tile_pool(name="op", bufs=len(sgroups)) as op,
            tc.tile_pool(name="ps", bufs=npt, space="PSUM") as ps,
        ):
            ones = bp.tile([1, 64], dt)
            nc.gpsimd.memset(ones[:], 1.0)
            # bz via the Pool engine's SWDGE path: desc-gen runs on the (idle)
            # Pool engine instead of the shared HWDGE that paces the loads
            bz = bp.tile([1, S * 64], dt)
            nc.gpsimd.dma_start(bz[:], bz_d[:])

            # loads on one engine in strict stream order (engine SEQ and the
            # shared HWDGE process them in order)
            wts, xts = [], []
            for c in range(nch):
                s_lo, s_hi = cb[c], cb[c + 1]
                cw = (s_hi - s_lo) * 64
                wt = wp.tile([128, KC, cw], dt, name=f"wt{c}")
                nc.sync.dma_start(wt[:], wt_d[:, s_lo * KC * 64 : s_hi * KC * 64])
                xt = xp.tile([128, (s_hi - s_lo) * KC * 64], dt, name=f"xt{c}")
                nc.sync.dma_start(xt[:], xt_d[:, s_lo * KC * 64 : s_hi * KC * 64])
                wts.append(wt)
                xts.append(xt)

            obs = []
            for gi, (t0, t1) in enumerate(sgroups):
                obs.append(op.tile([64, (pb[t1] - pb[t0]) * 64], dt, name=f"ob{gi}"))

            def chunk_of(sl):
                for c in range(nch):
                    if cb[c] <= sl < cb[c + 1]:
                        return c

            for gi, (t0, t1) in enumerate(sgroups):
                ob = obs[gi]
                for t in range(t0, t1):
                    p_lo, p_hi = pb[t], pb[t + 1]
                    tw = (p_hi - p_lo) * 64
                    acc = ps.tile([64, 512], mybir.dt.float32)
                    for sl in range(p_lo, p_hi):
                        c = chunk_of(sl)
                        o64 = (sl - p_lo) * 64
                        xo = (sl - cb[c]) * KC * 64
                        wo = (sl - cb[c]) * 64
                        for k in range(KC):
                            nc.tensor.matmul(
                                acc[0:64, o64 : o64 + 64],
                                xts[c][:, xo + k * 64 : xo + (k + 1) * 64],
                                wts[c][:, k, wo : wo + 64],
                                start=(sl == p_lo and k == 0),
                                stop=False,
                            )
                    # bias last so bz is never on the early critical path
                    nc.tensor.matmul(
                        acc[0:64, 0:tw],
                        ones[0:1, 0:64],
                        bz[0:1, p_lo * 64 : p_hi * 64],
                        start=False,
                        stop=True,
                    )
                    # psum drains: big tiles on DVE, the two small tail tiles
                    # on Act so they overlap DVE's last big copy
                    dst = ob[0:64, (p_lo - pb[t0]) * 64 : (p_lo - pb[t0]) * 64 + tw]
                    if t >= npt - 2:
                        nc.scalar.activation(
                            dst, acc[0:64, 0:tw], mybir.ActivationFunctionType.Copy
                        )
                    else:
                        nc.vector.tensor_copy(dst, acc[0:64, 0:tw])
                g_lo, g_hi = pb[t0] * 64, pb[t1] * 64
                # final (tiny) store on sync, whose SEQ is idle by then;
                # earlier stores on scalar, overlapped with the load stream
                eng = nc.sync if gi == len(sgroups) - 1 else nc.scalar
                eng.dma_start(o_d[:, g_lo:g_hi], ob[:])

    nc.compile()
    return nc


def _host_prep(x, W, bias, slots, S, goff):
    """Build xt/wt/bz arrays for one core."""
    nsl = len(slots)
    groups = np.array([g for g, _, _ in slots], np.int64)
    # xt: [128, S*KC*64]; col = s*(KC*64) + k*64 + b
    xg = x[:, goff + groups, :]                      # [B, nsl, H]
    xt = np.zeros((128, S * KC * 64), ml_dtypes.bfloat16)
    xt[:, : nsl * KC * 64] = (
        xg.reshape(B, nsl, KC, 128).transpose(3, 1, 2, 0).reshape(128, nsl * KC * 64)
    )
    # wt chunk-major: per chunk c the block [128, KC, chunk_cols] flattened at
    # col offset s_lo*KC*64
    Wp = np.zeros((S * 64, H), ml_dtypes.bfloat16)
    bz = np.zeros((1, S * 64), ml_dtypes.bfloat16)
    for s, (g, cst, wdt) in enumerate(slots):
        Wp[s * 64 : s * 64 + wdt] = W[cst : cst + wdt]
        bz[0, s * 64 : s * 64 + wdt] = bias[cst : cst + wdt]
    wt = np.zeros((128, KC * S * 64), ml_dtypes.bfloat16)
    cb = _chunk_bounds(S)
    for c in range(len(cb) - 1):
        s_lo, s_hi = cb[c], cb[c + 1]
        cw = (s_hi - s_lo) * 64
        blk = Wp[s_lo * 64 : s_hi * 64].reshape(cw, KC, 128).transpose(2, 1, 0)
        wt[:, s_lo * KC * 64 : s_hi * KC * 64] = blk.reshape(128, KC * cw)
    return {"xt": xt, "wt": wt, "bz": bz}


def kernel(x, co_W, cl_W, co_b, cl_b, co_group_of, cl_group_of, co_index,
           cl_index, group_len):
    x = np.asarray(x, np.float32)
    G = int(group_len)
    shards = _build_shards(co_group_of, cl_group_of)
    S = max(len(sl) for _, _, sl in shards)

    key = ("v7", S)
    if key not in _cache:
        _cache[key] = _program(S)
    nc = _cache[key]

    Ws = (np.asarray(co_W, np.float32)[0], np.asarray(cl_W, np.float32)[0])
    bs = (np.asarray(co_b, np.float32)[0], np.asarray(cl_b, np.float32)[0])
    in_maps = []
    for k in range(8):
        bi = k // NQ
        _, _, slots = shards[k]
        in_maps.append(_host_prep(x, Ws[bi], bs[bi], slots, S, bi * G))

    res = run_bass_kernel_spmd(nc, in_maps, list(range(8)))

    outs = []
    for bi, index in ((0, co_index), (1, cl_index)):
        full = np.empty((B, NC_CLS), np.float32)
        for q in range(NQ):
            r0, r1, slots = shards[bi * NQ + q]
            src = np.empty(r1 - r0, np.int64)
            for s, (g, cst, wdt) in enumerate(slots):
                src[cst - r0 : cst - r0 + wdt] = np.arange(s * 64, s * 64 + wdt)
            flat = np.asarray(res.results[bi * NQ + q]["o"]).astype(np.float32)
            full[:, r0:r1] = flat[:, src]
        outs.append(full[:, np.asarray(index).astype(np.int64)])
    return outs[0], outs[1]


# revision 38
# speedup vs baseline: 1.0334x; 1.0334x over previous
"""GroupWiseLinear Trainium2 kernel.

out[b, c] = dot(W[0, c, :], x[b, group_of[c], :]) + bias[0, c], then a final
class-permutation gather, for two independent branches (co / cl).

Sharding: 8 cores = 2 branches x 4 class-ranges.  The ranges are chosen per
branch to BALANCE the per-core slot count (ragged group segments are split
into <=64-column slots; fixed quarters give 23..30 slots, balanced ranges
give <=26), which directly cuts the dominant DMA traffic.  Every core runs
the SAME instruction stream (SPMD) on different data:

  - xt:  [128, S*4*64]   per-slot x^T (H-major), replicated per slot
  - wt:  [128, 4*S*64]   W^T, zero-padded to slot layout, chunk-major so
                         every load chunk is one contiguous line/partition
  - bz:  [1, S*64]       bias, zero-padded to slot layout
  - o:   [64, S*64]      bf16 per-core output (batch-major)

Device work per slot: 4 K-chunk matmuls (x stationary [128,64], W moving)
accumulating into PSUM, closed by a per-psum-tile rank-1 bias matmul
(ones x bias row, so bias data is never on the early critical path).  PSUM
drains to SBUF as bf16: big tiles on the vector engine, the two small tail
tiles on the activation engine so they overlap.  Loads stream on one engine
in strict order as paired (wt, xt) chunks sized [4, 8, .., 4, 1, 1]; psum
tiles are [8, .., 8, 1, 1] and stores are {t0,t1} / {t2,t3} / {t4}, so the
chain after the last loaded byte (sem prop -> 5 matmuls -> small copy ->
tiny store -> sem prop -> exit barrier) is as short as the cost model
allows.  bz rides the Pool engine's SWDGE path to keep the shared HWDGE
descriptor generator (625-650 ns per DMA, serialized) feeding the loads.
"""

import ml_dtypes
import numpy as np

import concourse.bacc as bacc
import concourse.tile as tile
from concourse import mybir
from concourse.bass_utils import run_bass_kernel_spmd

B = 64          # batch
H = 512         # hidden
NC_CLS = 4096   # classes per branch
NQ = 4          # class-ranges per branch
KC = H // 128   # contraction chunks

_cache = {}


def _segments(go, lo, hi):
    """Maximal same-group runs of go within [lo, hi)."""
    segs = []
    i = lo
    while i < hi:
        g = go[i]
        j = i
        while j < hi and go[j] == g:
            j += 1
        segs.append((i, j, int(g)))
        i = j
    return segs


def _balanced_split(go, nparts=NQ):
    """Split [0, len(go)) into nparts ranges with near-equal slot counts."""
    segs = _segments(go, 0, len(go))
    total = sum((e - s + 63) // 64 for s, e, _ in segs)
    target = -(-total // nparts)
    cuts = [0]
    acc = 0
    for s, e, _ in segs:
        n = (e - s + 63) // 64
        while acc + n >= target and len(cuts) < nparts:
            take = target - acc
            cutpos = min(s + take * 64, e)
            cuts.append(cutpos)
            s = cutpos
            n = (e - s + 63) // 64
            acc = 0
        acc += n
    cuts.append(len(go))
    return list(zip(cuts[:-1], cuts[1:]))


NPAIR = 4       # leading slot pairs that share one x tile


def _tile_of(s):
    """Static slot -> x-tile map: slots 0..2*NPAIR-1 pair up, rest single."""
    return s // 2 if s < 2 * NPAIR else s - NPAIR


def _build_shards(co_group_of, cl_group_of):
    """Per (branch, range): (r0, r1, slots) with slots (group, cls_start, w).
    Slots are reordered so the first 2*NPAIR are NPAIR same-group pairs
    (each pair shares one x tile via _tile_of); order is otherwise free
    because the unshard maps each slot to its class range explicitly."""
    shards = []
    for go in (co_group_of, cl_group_of):
        go = np.asarray(go).astype(np.int64)
        for r0, r1 in _balanced_split(go):
            runs = []
            for s0, s1, g in _segments(go, r0, r1):
                runs.append([(g, s, min(64, s1 - s)) for s in range(s0, s1, 64)])
            pairs, npair = [], 0
            for run in runs:
                while len(run) >= 2 and npair < NPAIR:
                    pairs += [run.pop(0), run.pop(0)]
                    npair += 1
            assert npair == NPAIR, f"only {npair} same-group slot pairs available"
            slots = pairs + [sl for run in runs for sl in run]
            shards.append((r0, r1, slots))
    return shards


def _chunk_bounds(S):
    """Slot-index bounds for paired (wt, xt) load chunks: a small first chunk
    so compute starts early, 8-slot middle chunks, and two single-slot final
    chunks so the tail of the pipeline is short."""
    bounds = [0, 4]
    while bounds[-1] < S - 2:
        bounds.append(min(bounds[-1] + 8, S - 2))
    bounds.append(S - 1)
    bounds.append(S)
    return bounds


def _ptile_bounds(S):
    """PSUM-tile bounds: 8 slots (one bank) each, single-slot final tile."""
    bounds = list(range(0, S - 1, 8))
    if bounds[-1] != S - 1:
        bounds.append(S - 1)
    bounds.append(S)
    return bounds


def _program(S, dt=mybir.dt.bfloat16):
    """Build the uniform SPMD Bass program for S slots per core."""
    nc = bacc.Bacc("TRN2", target_bir_lowering=False, debug=False, num_devices=8)
    xt_d = nc.dram_tensor("xt", [128, (S - NPAIR) * KC * 64], dt, kind="ExternalInput")
    wt_d = nc.dram_tensor("wt", [128, KC * S * 64], dt, kind="ExternalInput")
    bz_d = nc.dram_tensor("bz", [1, S * 64], dt, kind="ExternalInput")
    o_d = nc.dram_tensor("o", [64, S * 64], dt, kind="ExternalOutput")

    cb = _chunk_bounds(S)
    nch = len(cb) - 1
    pb = _ptile_bounds(S)
    npt = len(pb) - 1
    # store groups of psum tiles: pairs, with the small last tile alone so
    # the final store (and its whole chain) is tiny
    sgroups = []
    i = 0
    while i < npt - 1:
        j = min(i + 2, npt - 1)
        sgroups.append((i, j))
        i = j
    sgroups.append((npt - 1, npt))

    with tile.TileContext(nc) as tc:
        with (
            tc.tile_pool(name="bp", bufs=2) as bp,
            tc.tile_pool(name="wp", bufs=nch) as wp,
            tc.tile_pool(name="xp", bufs=nch) as xp,
            tc.tile_pool(name="op", bufs=len(sgroups)) as op,
            tc.tile_pool(name="ps", bufs=npt, space="PSUM") as ps,
        ):
            ones = bp.tile([1, 64], dt)
            nc.gpsimd.memset(ones[:], 1.0)
            # bz via the Pool engine's SWDGE path: desc-gen runs on the (idle)
            # Pool engine instead of the shared HWDGE that paces the loads
            bz = bp.tile([1, S * 64], dt)
            nc.gpsimd.dma_start(bz[:], bz_d[:])

            # loads on one engine in strict stream order (engine SEQ and the
            # shared HWDGE process them in order)
            wts, xts = [], []
            for c in range(nch):
                s_lo, s_hi = cb[c], cb[c + 1]
                cw = (s_hi - s_lo) * 64
                wt = wp.tile([128, KC, cw], dt, name=f"wt{c}")
                nc.sync.dma_start(wt[:], wt_d[:, s_lo * KC * 64 : s_hi * KC * 64])
                t_lo, t_hi = _tile_of(s_lo), _tile_of(s_hi)
                xt = xp.tile([128, (t_hi - t_lo) * KC * 64], dt, name=f"xt{c}")
                nc.sync.dma_start(xt[:], xt_d[:, t_lo * KC * 64 : t_hi * KC * 64])
                wts.append(wt)
                xts.append(xt)

            obs = []
            for gi, (t0, t1) in enumerate(sgroups):
                obs.append(op.tile([64, (pb[t1] - pb[t0]) * 64], dt, name=f"ob{gi}"))

            def chunk_of(sl):
                for c in range(nch):
                    if cb[c] <= sl < cb[c + 1]:
                        return c

            for gi, (t0, t1) in enumerate(sgroups):
                ob = obs[gi]
                for t in range(t0, t1):
                    p_lo, p_hi = pb[t], pb[t + 1]
                    tw = (p_hi - p_lo) * 64
                    acc = ps.tile([64, 512], mybir.dt.float32)
                    for sl in range(p_lo, p_hi):
                        c = chunk_of(sl)
                        o64 = (sl - p_lo) * 64
                        xo = (_tile_of(sl) - _tile_of(cb[c])) * KC * 64
                        wo = (sl - cb[c]) * 64
                        for k in range(KC):
                            nc.tensor.matmul(
                                acc[0:64, o64 : o64 + 64],
                                xts[c][:, xo + k * 64 : xo + (k + 1) * 64],
                                wts[c][:, k, wo : wo + 64],
                                start=(sl == p_lo and k == 0),
                                stop=False,
                            )
                    # bias last so bz is never on the early critical path
                    nc.tensor.matmul(
                        acc[0:64, 0:tw],
                        ones[0:1, 0:64],
                        bz[0:1, p_lo * 64 : p_hi * 64],
                        start=False,
                        stop=True,
                    )
                    # psum drains: big tiles on DVE, the two small tail tiles
                    # on Act so they overlap DVE's last big copy
                    dst = ob[0:64, (p_lo - pb[t0]) * 64 : (p_lo - pb[t0]) * 64 + tw]
                    if t >= npt - 2:
                        nc.scalar.activation(
                            dst, acc[0:64, 0:tw], mybir.ActivationFunctionType.Copy
                        )
                    else:
                        nc.vector.tensor_copy(dst, acc[0:64, 0:tw])
                g_lo, g_hi = pb[t0] * 64, pb[t1] * 64
                # final (tiny) store on sync, whose SEQ is idle by then;
                # earlier stores on scalar, overlapped with the load stream
                eng = nc.sync if gi == len(sgroups) - 1 else nc.scalar
                eng.dma_start(o_d[:, g_lo:g_hi], ob[:])

    nc.compile()
    return nc


def _host_prep(x, W, bias, slots, S, goff):
    """Build xt/wt/bz arrays for one core."""
    nsl = len(slots)
    groups = np.array([g for g, _, _ in slots], np.int64)
    # xt: one [128, KC*64] tile per x-tile; paired slots share a tile
    nt = S - NPAIR
    tile_groups = np.zeros(nt, np.int64)
    for s in range(min(nsl, S)):
        tile_groups[_tile_of(s)] = groups[s]
    xg = x[:, goff + tile_groups, :]                 # [B, nt, H]
    xt = np.ascontiguousarray(
        xg.reshape(B, nt, KC, 128).transpose(3, 1, 2, 0).reshape(128, nt * KC * 64),
        dtype=ml_dtypes.bfloat16,
    )
    # wt chunk-major: per chunk c the block [128, KC, chunk_cols] flattened at
    # col offset s_lo*KC*64
    Wp = np.zeros((S * 64, H), ml_dtypes.bfloat16)
    bz = np.zeros((1, S * 64), ml_dtypes.bfloat16)
    for s, (g, cst, wdt) in enumerate(slots):
        Wp[s * 64 : s * 64 + wdt] = W[cst : cst + wdt]
        bz[0, s * 64 : s * 64 + wdt] = bias[cst : cst + wdt]
    wt = np.zeros((128, KC * S * 64), ml_dtypes.bfloat16)
    cb = _chunk_bounds(S)
    for c in range(len(cb) - 1):
        s_lo, s_hi = cb[c], cb[c + 1]
        cw = (s_hi - s_lo) * 64
        blk = Wp[s_lo * 64 : s_hi * 64].reshape(cw, KC, 128).transpose(2, 1, 0)
        wt[:, s_lo * KC * 64 : s_hi * KC * 64] = blk.reshape(128, KC * cw)
    return {"xt": xt, "wt": wt, "bz": bz}


def kernel(x, co_W, cl_W, co_b, cl_b, co_group_of, cl_group_of, co_index,
           cl_index, group_len):
    x = np.asarray(x, np.float32)
    G = int(group_len)
    shards = _build_shards(co_group_of, cl_group_of)
    S = max(len(sl) for _, _, sl in shards)

    key = ("v10", S)
    if key not in _cache:
        _cache[key] = _program(S)
    nc = _cache[key]

    Ws = (np.asarray(co_W, np.float32)[0], np.asarray(cl_W, np.float32)[0])
    bs = (np.asarray(co_b, np.float32)[0], np.asarray(cl_b, np.float32)[0])
    in_maps = []
    for k in range(8):
        bi = k // NQ
        _, _, slots = shards[k]
        in_maps.append(_host_prep(x, Ws[bi], bs[bi], slots, S, bi * G))

    res = run_bass_kernel_spmd(nc, in_maps, list(range(8)))

    outs = []
    for bi, index in ((0, co_index), (1, cl_index)):
        full = np.empty((B, NC_CLS), np.float32)
        for q in range(NQ):
            r0, r1, slots = shards[bi * NQ + q]
            src = np.empty(r1 - r0, np.int64)
            for s, (g, cst, wdt) in enumerate(slots):
                src[cst - r0 : cst - r0 + wdt] = np.arange(s * 64, s * 64 + wdt)
            flat = np.asarray(res.results[bi * NQ + q]["o"]).astype(np.float32)
            full[:, r0:r1] = flat[:, src]
        outs.append(full[:, np.asarray(index).astype(np.int64)])
    return outs[0], outs[1]


# revision 39
# speedup vs baseline: 1.0452x; 1.0114x over previous
"""GroupWiseLinear Trainium2 kernel.

out[b, c] = dot(W[0, c, :], x[b, group_of[c], :]) + bias[0, c], then a final
class-permutation gather, for two independent branches (co / cl).

Sharding: 8 cores = 2 branches x 4 class-ranges.  The ranges are chosen per
branch to BALANCE the per-core slot count (ragged group segments are split
into <=64-column slots; fixed quarters give 23..30 slots, balanced ranges
give <=26), which directly cuts the dominant DMA traffic.  Every core runs
the SAME instruction stream (SPMD) on different data:

  - xt:  [128, S*4*64]   per-slot x^T (H-major), replicated per slot
  - wt:  [128, 4*S*64]   W^T, zero-padded to slot layout, chunk-major so
                         every load chunk is one contiguous line/partition
  - bz:  [1, S*64]       bias, zero-padded to slot layout
  - o:   [64, S*64]      bf16 per-core output (batch-major)

Device work per slot: 4 K-chunk matmuls (x stationary [128,64], W moving)
accumulating into PSUM, closed by a per-psum-tile rank-1 bias matmul
(ones x bias row, so bias data is never on the early critical path).  PSUM
drains to SBUF as bf16: big tiles on the vector engine, the two small tail
tiles on the activation engine so they overlap.  Loads stream on one engine
in strict order as paired (wt, xt) chunks sized [4, 8, .., 4, 1, 1]; psum
tiles are [8, .., 8, 1, 1] and stores are {t0,t1} / {t2,t3} / {t4}, so the
chain after the last loaded byte (sem prop -> 5 matmuls -> small copy ->
tiny store -> sem prop -> exit barrier) is as short as the cost model
allows.  bz rides the Pool engine's SWDGE path to keep the shared HWDGE
descriptor generator (625-650 ns per DMA, serialized) feeding the loads.
"""

import ml_dtypes
import numpy as np

import concourse.bacc as bacc
import concourse.tile as tile
from concourse import mybir
from concourse.bass_utils import run_bass_kernel_spmd

B = 64          # batch
H = 512         # hidden
NC_CLS = 4096   # classes per branch
NQ = 4          # class-ranges per branch
KC = H // 128   # contraction chunks

_cache = {}


def _segments(go, lo, hi):
    """Maximal same-group runs of go within [lo, hi)."""
    segs = []
    i = lo
    while i < hi:
        g = go[i]
        j = i
        while j < hi and go[j] == g:
            j += 1
        segs.append((i, j, int(g)))
        i = j
    return segs


def _range_stats(go, r0, r1):
    """(slots, same-group pairs) of range [r0, r1)."""
    S = P = 0
    i = r0
    while i < r1:
        g = go[i]
        j = i
        while j < r1 and go[j] == g:
            j += 1
        n = (j - i + 63) // 64
        S += n
        P += n // 2
        i = j
    return S, P


def _balanced_split(go, nparts=NQ):
    """Split [0, len(go)) into nparts ranges with near-equal slot counts,
    preferring splits where every range also has >= NPAIR same-group slot
    pairs (so each pair can share one x tile)."""
    segs = _segments(go, 0, len(go))
    total = sum((e - s + 63) // 64 for s, e, _ in segs)
    target = -(-total // nparts)
    # plain greedy balance first
    cuts = [0]
    acc = 0
    for s, e, _ in segs:
        n = (e - s + 63) // 64
        while acc + n >= target and len(cuts) < nparts:
            take = target - acc
            cutpos = min(s + take * 64, e)
            cuts.append(cutpos)
            s = cutpos
            n = (e - s + 63) // 64
            acc = 0
        acc += n
    cuts.append(len(go))
    parts = list(zip(cuts[:-1], cuts[1:]))
    if all(_range_stats(go, r0, r1)[1] >= NPAIR for r0, r1 in parts):
        return parts
    # bounded search over 64-aligned candidate cuts for pair-rich splits
    C = len(go)
    cands = sorted(
        {min(s + k, e) for s, e, _ in segs for k in range(0, e - s + 64, 64)}
        - {0, C}
    )
    qw = C // nparts
    def feas(r0, r1):
        S, P = _range_stats(go, r0, r1)
        return S <= target and P >= NPAIR
    for c1 in (c for c in cands if abs(c - qw) < qw // 2):
        if not feas(0, c1):
            continue
        for c2 in (c for c in cands if c1 < c and abs(c - 2 * qw) < qw // 2):
            if not feas(c1, c2):
                continue
            for c3 in (c for c in cands if c2 < c and abs(c - 3 * qw) < qw // 2):
                if feas(c2, c3) and feas(c3, C):
                    return [(0, c1), (c1, c2), (c2, c3), (c3, C)]
    return parts  # fall back (caller's assert will trip if pairs short)


NPAIR = 5       # leading slot pairs that share one x tile


def _tile_of(s):
    """Static slot -> x-tile map: slots 0..2*NPAIR-1 pair up, rest single."""
    return s // 2 if s < 2 * NPAIR else s - NPAIR


def _build_shards(co_group_of, cl_group_of):
    """Per (branch, range): (r0, r1, slots) with slots (group, cls_start, w).
    Slots are reordered so the first 2*NPAIR are NPAIR same-group pairs
    (each pair shares one x tile via _tile_of); order is otherwise free
    because the unshard maps each slot to its class range explicitly."""
    shards = []
    for go in (co_group_of, cl_group_of):
        go = np.asarray(go).astype(np.int64)
        for r0, r1 in _balanced_split(go):
            runs = []
            for s0, s1, g in _segments(go, r0, r1):
                runs.append([(g, s, min(64, s1 - s)) for s in range(s0, s1, 64)])
            pairs, npair = [], 0
            for run in runs:
                while len(run) >= 2 and npair < NPAIR:
                    pairs += [run.pop(0), run.pop(0)]
                    npair += 1
            assert npair == NPAIR, f"only {npair} same-group slot pairs available"
            slots = pairs + [sl for run in runs for sl in run]
            shards.append((r0, r1, slots))
    return shards


def _chunk_bounds(S):
    """Slot-index bounds for paired (wt, xt) load chunks: a small first chunk
    so compute starts early, 8-slot middle chunks, and two single-slot final
    chunks so the tail of the pipeline is short."""
    bounds = [0, 4]
    while bounds[-1] < S - 2:
        bounds.append(min(bounds[-1] + 8, S - 2))
    bounds.append(S - 1)
    bounds.append(S)
    return bounds


def _ptile_bounds(S):
    """PSUM-tile bounds: 8 slots (one bank) each, single-slot final tile."""
    bounds = list(range(0, S - 1, 8))
    if bounds[-1] != S - 1:
        bounds.append(S - 1)
    bounds.append(S)
    return bounds


def _program(S, dt=mybir.dt.bfloat16):
    """Build the uniform SPMD Bass program for S slots per core."""
    nc = bacc.Bacc("TRN2", target_bir_lowering=False, debug=False, num_devices=8)
    xt_d = nc.dram_tensor("xt", [128, (S - NPAIR) * KC * 64], dt, kind="ExternalInput")
    wt_d = nc.dram_tensor("wt", [128, KC * S * 64], dt, kind="ExternalInput")
    bz_d = nc.dram_tensor("bz", [1, S * 64], dt, kind="ExternalInput")
    o_d = nc.dram_tensor("o", [64, S * 64], dt, kind="ExternalOutput")

    cb = _chunk_bounds(S)
    nch = len(cb) - 1
    pb = _ptile_bounds(S)
    npt = len(pb) - 1
    # store groups of psum tiles: pairs, with the small last tile alone so
    # the final store (and its whole chain) is tiny
    sgroups = []
    i = 0
    while i < npt - 1:
        j = min(i + 2, npt - 1)
        sgroups.append((i, j))
        i = j
    sgroups.append((npt - 1, npt))

    with tile.TileContext(nc) as tc:
        with (
            tc.tile_pool(name="bp", bufs=2) as bp,
            tc.tile_pool(name="wp", bufs=nch) as wp,
            tc.tile_pool(name="xp", bufs=nch) as xp,
            tc.tile_pool(name="op", bufs=len(sgroups)) as op,
            tc.tile_pool(name="ps", bufs=npt, space="PSUM") as ps,
        ):
            ones = bp.tile([1, 64], dt)
            nc.gpsimd.memset(ones[:], 1.0)
            # bz via the Pool engine's SWDGE path: desc-gen runs on the (idle)
            # Pool engine instead of the shared HWDGE that paces the loads
            bz = bp.tile([1, S * 64], dt)
            nc.gpsimd.dma_start(bz[:], bz_d[:])

            # loads on one engine in strict stream order (engine SEQ and the
            # shared HWDGE process them in order)
            wts, xts = [], []
            for c in range(nch):
                s_lo, s_hi = cb[c], cb[c + 1]
                cw = (s_hi - s_lo) * 64
                wt = wp.tile([128, KC, cw], dt, name=f"wt{c}")
                nc.sync.dma_start(wt[:], wt_d[:, s_lo * KC * 64 : s_hi * KC * 64])
                t_lo, t_hi = _tile_of(s_lo), _tile_of(s_hi)
                xt = xp.tile([128, (t_hi - t_lo) * KC * 64], dt, name=f"xt{c}")
                nc.sync.dma_start(xt[:], xt_d[:, t_lo * KC * 64 : t_hi * KC * 64])
                wts.append(wt)
                xts.append(xt)

            obs = []
            for gi, (t0, t1) in enumerate(sgroups):
                obs.append(op.tile([64, (pb[t1] - pb[t0]) * 64], dt, name=f"ob{gi}"))

            def chunk_of(sl):
                for c in range(nch):
                    if cb[c] <= sl < cb[c + 1]:
                        return c

            for gi, (t0, t1) in enumerate(sgroups):
                ob = obs[gi]
                for t in range(t0, t1):
                    p_lo, p_hi = pb[t], pb[t + 1]
                    tw = (p_hi - p_lo) * 64
                    acc = ps.tile([64, 512], mybir.dt.float32)
                    for sl in range(p_lo, p_hi):
                        c = chunk_of(sl)
                        o64 = (sl - p_lo) * 64
                        xo = (_tile_of(sl) - _tile_of(cb[c])) * KC * 64
                        wo = (sl - cb[c]) * 64
                        for k in range(KC):
                            nc.tensor.matmul(
                                acc[0:64, o64 : o64 + 64],
                                xts[c][:, xo + k * 64 : xo + (k + 1) * 64],
                                wts[c][:, k, wo : wo + 64],
                                start=(sl == p_lo and k == 0),
                                stop=False,
                            )
                    # bias last so bz is never on the early critical path
                    nc.tensor.matmul(
                        acc[0:64, 0:tw],
                        ones[0:1, 0:64],
                        bz[0:1, p_lo * 64 : p_hi * 64],
                        start=False,
                        stop=True,
                    )
                    # psum drains: big tiles on DVE, the two small tail tiles
                    # on Act so they overlap DVE's last big copy
                    dst = ob[0:64, (p_lo - pb[t0]) * 64 : (p_lo - pb[t0]) * 64 + tw]
                    if t >= npt - 2:
                        nc.scalar.activation(
                            dst, acc[0:64, 0:tw], mybir.ActivationFunctionType.Copy
                        )
                    else:
                        nc.vector.tensor_copy(dst, acc[0:64, 0:tw])
                g_lo, g_hi = pb[t0] * 64, pb[t1] * 64
                # final (tiny) store on sync, whose SEQ is idle by then;
                # earlier stores on scalar, overlapped with the load stream
                eng = nc.sync if gi == len(sgroups) - 1 else nc.scalar
                eng.dma_start(o_d[:, g_lo:g_hi], ob[:])

    nc.compile()
    return nc


def _host_prep(x, W, bias, slots, S, goff):
    """Build xt/wt/bz arrays for one core."""
    nsl = len(slots)
    groups = np.array([g for g, _, _ in slots], np.int64)
    # xt: one [128, KC*64] tile per x-tile; paired slots share a tile
    nt = S - NPAIR
    tile_groups = np.zeros(nt, np.int64)
    for s in range(min(nsl, S)):
        tile_groups[_tile_of(s)] = groups[s]
    xg = x[:, goff + tile_groups, :]                 # [B, nt, H]
    xt = np.ascontiguousarray(
        xg.reshape(B, nt, KC, 128).transpose(3, 1, 2, 0).reshape(128, nt * KC * 64),
        dtype=ml_dtypes.bfloat16,
    )
    # wt chunk-major: per chunk c the block [128, KC, chunk_cols] flattened at
    # col offset s_lo*KC*64
    Wp = np.zeros((S * 64, H), ml_dtypes.bfloat16)
    bz = np.zeros((1, S * 64), ml_dtypes.bfloat16)
    for s, (g, cst, wdt) in enumerate(slots):
        Wp[s * 64 : s * 64 + wdt] = W[cst : cst + wdt]
        bz[0, s * 64 : s * 64 + wdt] = bias[cst : cst + wdt]
    wt = np.zeros((128, KC * S * 64), ml_dtypes.bfloat16)
    cb = _chunk_bounds(S)
    for c in range(len(cb) - 1):
        s_lo, s_hi = cb[c], cb[c + 1]
        cw = (s_hi - s_lo) * 64
        blk = Wp[s_lo * 64 : s_hi * 64].reshape(cw, KC, 128).transpose(2, 1, 0)
        wt[:, s_lo * KC * 64 : s_hi * KC * 64] = blk.reshape(128, KC * cw)
    return {"xt": xt, "wt": wt, "bz": bz}


def kernel(x, co_W, cl_W, co_b, cl_b, co_group_of, cl_group_of, co_index,
           cl_index, group_len):
    x = np.asarray(x, np.float32)
    G = int(group_len)
    shards = _build_shards(co_group_of, cl_group_of)
    S = max(len(sl) for _, _, sl in shards)

    key = ("v11", S)
    if key not in _cache:
        _cache[key] = _program(S)
    nc = _cache[key]

    Ws = (np.asarray(co_W, np.float32)[0], np.asarray(cl_W, np.float32)[0])
    bs = (np.asarray(co_b, np.float32)[0], np.asarray(cl_b, np.float32)[0])
    in_maps = []
    for k in range(8):
        bi = k // NQ
        _, _, slots = shards[k]
        in_maps.append(_host_prep(x, Ws[bi], bs[bi], slots, S, bi * G))

    res = run_bass_kernel_spmd(nc, in_maps, list(range(8)))

    outs = []
    for bi, index in ((0, co_index), (1, cl_index)):
        full = np.empty((B, NC_CLS), np.float32)
        for q in range(NQ):
            r0, r1, slots = shards[bi * NQ + q]
            src = np.empty(r1 - r0, np.int64)
            for s, (g, cst, wdt) in enumerate(slots):
                src[cst - r0 : cst - r0 + wdt] = np.arange(s * 64, s * 64 + wdt)
            flat = np.asarray(res.results[bi * NQ + q]["o"]).astype(np.float32)
            full[:, r0:r1] = flat[:, src]
        outs.append(full[:, np.asarray(index).astype(np.int64)])
    return outs[0], outs[1]


# revision 40
# speedup vs baseline: 1.0573x; 1.0116x over previous
"""GroupWiseLinear Trainium2 kernel.

out[b, c] = dot(W[0, c, :], x[b, group_of[c], :]) + bias[0, c], then a final
class-permutation gather, for two independent branches (co / cl).

Sharding: 8 cores = 2 branches x 4 class-ranges.  The ranges are chosen per
branch to BALANCE the per-core slot count (ragged group segments are split
into <=64-column slots; fixed quarters give 23..30 slots, balanced ranges
give <=26), which directly cuts the dominant DMA traffic.  Every core runs
the SAME instruction stream (SPMD) on different data:

  - xt:  [128, S*4*64]   per-slot x^T (H-major), replicated per slot
  - wt:  [128, 4*S*64]   W^T, zero-padded to slot layout, chunk-major so
                         every load chunk is one contiguous line/partition
  - bz:  [1, S*64]       bias, zero-padded to slot layout
  - o:   [64, S*64]      bf16 per-core output (batch-major)

Device work per slot: 4 K-chunk matmuls (x stationary [128,64], W moving)
accumulating into PSUM, closed by a per-psum-tile rank-1 bias matmul
(ones x bias row, so bias data is never on the early critical path).  PSUM
drains to SBUF as bf16: big tiles on the vector engine, the two small tail
tiles on the activation engine so they overlap.  Loads stream on one engine
in strict order as paired (wt, xt) chunks sized [4, 8, .., 4, 1, 1]; psum
tiles are [8, .., 8, 1, 1] and stores are {t0,t1} / {t2,t3} / {t4}, so the
chain after the last loaded byte (sem prop -> 5 matmuls -> small copy ->
tiny store -> sem prop -> exit barrier) is as short as the cost model
allows.  bz rides the Pool engine's SWDGE path to keep the shared HWDGE
descriptor generator (625-650 ns per DMA, serialized) feeding the loads.
"""

import ml_dtypes
import numpy as np

import concourse.bacc as bacc
import concourse.tile as tile
from concourse import mybir
from concourse.bass_utils import run_bass_kernel_spmd

B = 64          # batch
H = 512         # hidden
NC_CLS = 4096   # classes per branch
NQ = 4          # class-ranges per branch
KC = H // 128   # contraction chunks

_cache = {}


def _segments(go, lo, hi):
    """Maximal same-group runs of go within [lo, hi)."""
    segs = []
    i = lo
    while i < hi:
        g = go[i]
        j = i
        while j < hi and go[j] == g:
            j += 1
        segs.append((i, j, int(g)))
        i = j
    return segs


def _range_runs(go, r0, r1):
    """(slots, per-segment slot counts) of range [r0, r1)."""
    S = 0
    runs = []
    i = r0
    while i < r1:
        g = go[i]
        j = i
        while j < r1 and go[j] == g:
            j += 1
        n = (j - i + 63) // 64
        S += n
        runs.append(n)
        i = j
    return S, runs


def _balanced_split(go, nparts=NQ):
    """Split [0, len(go)) into nparts ranges with near-equal slot counts,
    preferring splits where every range also has >= NPAIR same-group slot
    pairs (so each pair can share one x tile)."""
    segs = _segments(go, 0, len(go))
    total = sum((e - s + 63) // 64 for s, e, _ in segs)
    target = -(-total // nparts)
    # plain greedy balance first
    cuts = [0]
    acc = 0
    for s, e, _ in segs:
        n = (e - s + 63) // 64
        while acc + n >= target and len(cuts) < nparts:
            take = target - acc
            cutpos = min(s + take * 64, e)
            cuts.append(cutpos)
            s = cutpos
            n = (e - s + 63) // 64
            acc = 0
        acc += n
    cuts.append(len(go))
    parts = list(zip(cuts[:-1], cuts[1:]))
    def part_ok(r0, r1):
        _, runs = _range_runs(go, r0, r1)
        for k, n in enumerate(runs):
            if n >= 3 and sum(v // 2 for v in runs[:k] + [n - 3] + runs[k + 1:]) >= NPAIR:
                return True
        return False
    if all(part_ok(r0, r1) for r0, r1 in parts):
        return parts
    # bounded search over 64-aligned candidate cuts for pair-rich splits
    C = len(go)
    cands = sorted(
        {min(s + k, e) for s, e, _ in segs for k in range(0, e - s + 64, 64)}
        - {0, C}
    )
    qw = C // nparts
    def feas(r0, r1):
        S, runs = _range_runs(go, r0, r1)
        if S > target:
            return False
        for k, n in enumerate(runs):
            if n >= 3 and sum(v // 2 for v in runs[:k] + [n - 3] + runs[k + 1:]) >= NPAIR:
                return True
        return False
    for c1 in (c for c in cands if abs(c - qw) < qw // 2):
        if not feas(0, c1):
            continue
        for c2 in (c for c in cands if c1 < c and abs(c - 2 * qw) < qw // 2):
            if not feas(c1, c2):
                continue
            for c3 in (c for c in cands if c2 < c and abs(c - 3 * qw) < qw // 2):
                if feas(c2, c3) and feas(c3, C):
                    return [(0, c1), (c1, c2), (c2, c3), (c3, C)]
    return parts  # fall back (caller's assert will trip if pairs short)


NPAIR = 4       # leading slot pairs that share one x tile
NTRIP = 1       # one slot triple after the pairs shares one x tile
NSAVE = NPAIR + 2 * NTRIP


def _tile_of(s):
    """Static slot -> x-tile map: 4 pairs, then a triple, rest single."""
    if s < 8:
        return s // 2
    if s < 11:
        return 4
    return s - NSAVE


def _build_shards(co_group_of, cl_group_of):
    """Per (branch, range): (r0, r1, slots) with slots (group, cls_start, w).
    Slots are reordered so the first 2*NPAIR are NPAIR same-group pairs
    (each pair shares one x tile via _tile_of); order is otherwise free
    because the unshard maps each slot to its class range explicitly."""
    shards = []
    for go in (co_group_of, cl_group_of):
        go = np.asarray(go).astype(np.int64)
        for r0, r1 in _balanced_split(go):
            runs = []
            for s0, s1, g in _segments(go, r0, r1):
                runs.append([(g, s, min(64, s1 - s)) for s in range(s0, s1, 64)])
            # extract the triple from a >=3-slot segment whose removal
            # still leaves NPAIR pairs, then the pairs
            trip = None
            for k, run in enumerate(runs):
                if len(run) >= 3:
                    rem = [len(r) for r in runs[:k]] + [len(run) - 3] + [
                        len(r) for r in runs[k + 1:]]
                    if sum(v // 2 for v in rem) >= NPAIR:
                        trip = run[:3]
                        runs[k] = run[3:]
                        break
            assert trip is not None, "no feasible slot triple"
            pairs, npair = [], 0
            for run in runs:
                while len(run) >= 2 and npair < NPAIR:
                    pairs += [run.pop(0), run.pop(0)]
                    npair += 1
            assert npair == NPAIR, f"only {npair} same-group slot pairs available"
            slots = pairs + trip + [sl for run in runs for sl in run]
            shards.append((r0, r1, slots))
    return shards


def _chunk_bounds(S):
    """Slot-index bounds for paired (wt, xt) load chunks: a small first chunk
    so compute starts early, 8-slot middle chunks, and two single-slot final
    chunks so the tail of the pipeline is short."""
    bounds = [0, 4]
    while bounds[-1] < S - 2:
        bounds.append(min(bounds[-1] + 8, S - 2))
    bounds.append(S - 1)
    bounds.append(S)
    return bounds


def _ptile_bounds(S):
    """PSUM-tile bounds: 8 slots (one bank) each, single-slot final tile."""
    bounds = list(range(0, S - 1, 8))
    if bounds[-1] != S - 1:
        bounds.append(S - 1)
    bounds.append(S)
    return bounds


def _program(S, dt=mybir.dt.bfloat16):
    """Build the uniform SPMD Bass program for S slots per core."""
    nc = bacc.Bacc("TRN2", target_bir_lowering=False, debug=False, num_devices=8)
    xt_d = nc.dram_tensor("xt", [128, (S - NSAVE) * KC * 64], dt, kind="ExternalInput")
    wt_d = nc.dram_tensor("wt", [128, KC * S * 64], dt, kind="ExternalInput")
    bz_d = nc.dram_tensor("bz", [1, S * 64], dt, kind="ExternalInput")
    o_d = nc.dram_tensor("o", [64, S * 64], dt, kind="ExternalOutput")

    cb = _chunk_bounds(S)
    nch = len(cb) - 1
    pb = _ptile_bounds(S)
    npt = len(pb) - 1
    # store groups of psum tiles: pairs, with the small last tile alone so
    # the final store (and its whole chain) is tiny
    sgroups = []
    i = 0
    while i < npt - 1:
        j = min(i + 2, npt - 1)
        sgroups.append((i, j))
        i = j
    sgroups.append((npt - 1, npt))

    with tile.TileContext(nc) as tc:
        with (
            tc.tile_pool(name="bp", bufs=2) as bp,
            tc.tile_pool(name="wp", bufs=nch) as wp,
            tc.tile_pool(name="xp", bufs=nch) as xp,
            tc.tile_pool(name="op", bufs=len(sgroups)) as op,
            tc.tile_pool(name="ps", bufs=npt, space="PSUM") as ps,
        ):
            ones = bp.tile([1, 64], dt)
            nc.gpsimd.memset(ones[:], 1.0)
            # bz via the Pool engine's SWDGE path: desc-gen runs on the (idle)
            # Pool engine instead of the shared HWDGE that paces the loads
            bz = bp.tile([1, S * 64], dt)
            nc.gpsimd.dma_start(bz[:], bz_d[:])

            # loads on one engine in strict stream order (engine SEQ and the
            # shared HWDGE process them in order)
            wts, xts = [], []
            for c in range(nch):
                s_lo, s_hi = cb[c], cb[c + 1]
                cw = (s_hi - s_lo) * 64
                wt = wp.tile([128, KC, cw], dt, name=f"wt{c}")
                nc.sync.dma_start(wt[:], wt_d[:, s_lo * KC * 64 : s_hi * KC * 64])
                t_lo, t_hi = _tile_of(s_lo), _tile_of(s_hi)
                xt = xp.tile([128, (t_hi - t_lo) * KC * 64], dt, name=f"xt{c}")
                nc.sync.dma_start(xt[:], xt_d[:, t_lo * KC * 64 : t_hi * KC * 64])
                wts.append(wt)
                xts.append(xt)

            obs = []
            for gi, (t0, t1) in enumerate(sgroups):
                obs.append(op.tile([64, (pb[t1] - pb[t0]) * 64], dt, name=f"ob{gi}"))

            def chunk_of(sl):
                for c in range(nch):
                    if cb[c] <= sl < cb[c + 1]:
                        return c

            for gi, (t0, t1) in enumerate(sgroups):
                ob = obs[gi]
                for t in range(t0, t1):
                    p_lo, p_hi = pb[t], pb[t + 1]
                    tw = (p_hi - p_lo) * 64
                    acc = ps.tile([64, 512], mybir.dt.float32)
                    for sl in range(p_lo, p_hi):
                        c = chunk_of(sl)
                        o64 = (sl - p_lo) * 64
                        xo = (_tile_of(sl) - _tile_of(cb[c])) * KC * 64
                        wo = (sl - cb[c]) * 64
                        for k in range(KC):
                            nc.tensor.matmul(
                                acc[0:64, o64 : o64 + 64],
                                xts[c][:, xo + k * 64 : xo + (k + 1) * 64],
                                wts[c][:, k, wo : wo + 64],
                                start=(sl == p_lo and k == 0),
                                stop=False,
                            )
                    # bias last so bz is never on the early critical path
                    nc.tensor.matmul(
                        acc[0:64, 0:tw],
                        ones[0:1, 0:64],
                        bz[0:1, p_lo * 64 : p_hi * 64],
                        start=False,
                        stop=True,
                    )
                    # psum drains: big tiles on DVE, the two small tail tiles
                    # on Act so they overlap DVE's last big copy
                    dst = ob[0:64, (p_lo - pb[t0]) * 64 : (p_lo - pb[t0]) * 64 + tw]
                    if t >= npt - 2:
                        nc.scalar.activation(
                            dst, acc[0:64, 0:tw], mybir.ActivationFunctionType.Copy
                        )
                    else:
                        nc.vector.tensor_copy(dst, acc[0:64, 0:tw])
                g_lo, g_hi = pb[t0] * 64, pb[t1] * 64
                # final (tiny) store on sync, whose SEQ is idle by then;
                # earlier stores on scalar, overlapped with the load stream
                eng = nc.sync if gi == len(sgroups) - 1 else nc.scalar
                eng.dma_start(o_d[:, g_lo:g_hi], ob[:])

    nc.compile()
    return nc


def _host_prep(x, W, bias, slots, S, goff):
    """Build xt/wt/bz arrays for one core."""
    nsl = len(slots)
    groups = np.array([g for g, _, _ in slots], np.int64)
    # xt: one [128, KC*64] tile per x-tile; paired slots share a tile
    nt = S - NSAVE
    tile_groups = np.zeros(nt, np.int64)
    for s in range(min(nsl, S)):
        tile_groups[_tile_of(s)] = groups[s]
    xg = x[:, goff + tile_groups, :]                 # [B, nt, H]
    xt = np.ascontiguousarray(
        xg.reshape(B, nt, KC, 128).transpose(3, 1, 2, 0).reshape(128, nt * KC * 64),
        dtype=ml_dtypes.bfloat16,
    )
    # wt chunk-major: per chunk c the block [128, KC, chunk_cols] flattened at
    # col offset s_lo*KC*64
    Wp = np.zeros((S * 64, H), ml_dtypes.bfloat16)
    bz = np.zeros((1, S * 64), ml_dtypes.bfloat16)
    for s, (g, cst, wdt) in enumerate(slots):
        Wp[s * 64 : s * 64 + wdt] = W[cst : cst + wdt]
        bz[0, s * 64 : s * 64 + wdt] = bias[cst : cst + wdt]
    wt = np.zeros((128, KC * S * 64), ml_dtypes.bfloat16)
    cb = _chunk_bounds(S)
    for c in range(len(cb) - 1):
        s_lo, s_hi = cb[c], cb[c + 1]
        cw = (s_hi - s_lo) * 64
        blk = Wp[s_lo * 64 : s_hi * 64].reshape(cw, KC, 128).transpose(2, 1, 0)
        wt[:, s_lo * KC * 64 : s_hi * KC * 64] = blk.reshape(128, KC * cw)
    return {"xt": xt, "wt": wt, "bz": bz}


def kernel(x, co_W, cl_W, co_b, cl_b, co_group_of, cl_group_of, co_index,
           cl_index, group_len):
    x = np.asarray(x, np.float32)
    G = int(group_len)
    shards = _build_shards(co_group_of, cl_group_of)
    S = max(len(sl) for _, _, sl in shards)

    key = ("v12", S)
    if key not in _cache:
        _cache[key] = _program(S)
    nc = _cache[key]

    Ws = (np.asarray(co_W, np.float32)[0], np.asarray(cl_W, np.float32)[0])
    bs = (np.asarray(co_b, np.float32)[0], np.asarray(cl_b, np.float32)[0])
    in_maps = []
    for k in range(8):
        bi = k // NQ
        _, _, slots = shards[k]
        in_maps.append(_host_prep(x, Ws[bi], bs[bi], slots, S, bi * G))

    res = run_bass_kernel_spmd(nc, in_maps, list(range(8)))

    outs = []
    for bi, index in ((0, co_index), (1, cl_index)):
        full = np.empty((B, NC_CLS), np.float32)
        for q in range(NQ):
            r0, r1, slots = shards[bi * NQ + q]
            src = np.empty(r1 - r0, np.int64)
            for s, (g, cst, wdt) in enumerate(slots):
                src[cst - r0 : cst - r0 + wdt] = np.arange(s * 64, s * 64 + wdt)
            flat = np.asarray(res.results[bi * NQ + q]["o"]).astype(np.float32)
            full[:, r0:r1] = flat[:, src]
        outs.append(full[:, np.asarray(index).astype(np.int64)])
    return outs[0], outs[1]
